# revision 4
# baseline (speedup 1.0000x reference)
"""Trainium2 Bass kernel: one step of a 2D wave equation with PML damping.

Reference computation (single step, grid [4080, 4080], f32):
    p1c = p1 * pml_coeff
    p2c = p2 * pml_coeff
    v     = edge-pad(varray, 40)                      # [4080, 4080]
    alpha = v^2 * DT^2
    L[i,j] = p2c[i,j-1] + p2c[i,j+1] + p2c[i-1,j] + p2c[i+1,j] - 4*p2c[i,j]
    p[i,j] = 2*p2c[i,j] - p1c[i,j] + alpha[i,j] * L[i,j] / DX^2   (interior i,j in [8, 4072))
    p[40, 60] += source_function[t] * DT^2
    returns (p, p[40:4040, 90])

Sharding: rows split across 8 NeuronCores, 508 interior output rows per core,
with a 1-row halo on p1/p2/pml (stencil radius 1).  Per core the 5-point
Laplacian is computed on the PE with three accumulating matmuls (a banded
[1,-4,1] stencil matrix plus a row-shift selector), the non-alpha terms
(2*p2c - p1c) with two more matmuls into a second PSUM bank, and the
remaining elementwise work on DVE/ACT/GpSimd.  Compute-engine access
patterns must start at partition 0 on this toolchain, so the stencil
matrices encode the one-row shift between the halo-aligned input tiles
(partition k = output row r0+k-1) and the output tiles (partition m =
output row r0+m).
"""

import numpy as np

import concourse.bass as bass
import concourse.mybir as mybir
from concourse.bass_utils import run_bass_kernel_spmd
from concourse.tile import TileContext

# ---------------------------------------------------------------- constants
EXT = 4080          # extended grid (NX + 2*PML)
NX = 4000
PML = 40
DT = 0.0005
DX = 10.0
NCORES = 8
RPC = 508           # interior output rows per core (8*508 = 4064 = EXT-16)
BAND = 126          # output rows per full row-band (126*4 + 4 = 508)
NBANDS = 5
SBW = 2032          # output columns per superblock (2 * 2032 = 4064)
CH = 508            # output columns per compute chunk (4 per superblock)
F32 = mybir.dt.float32


def split_sync_waits(nc: bass.Bass) -> None:
    """Rewrite instructions so none carries more than one sync wait.

    The walrus build in this container rejects instructions with multiple
    sync-wait commands ("Too many sync wait commands"), but Tile's semaphore
    assignment freely attaches several.  Waiting on [w1..wn] at one
    instruction is equivalent to waiting on each in turn on the same engine,
    so hoist all but the last wait onto nops inserted just before.
    """
    for f in nc.m.functions:
        for blk in f.blocks:
            il = blk.instructions
            i = 0
            while i < len(il):
                inst = il[i]
                si = inst.sync_info
                if si is not None and len(si.on_wait) > 1:
                    waits = list(si.on_wait)
                    for j, w in enumerate(waits[:-1]):
                        nop = mybir.InstNoOp(
                            name=nc.get_next_instruction_name(),
                            engine=inst.engine,
                            bass_nofuse=True,
                            sync_info=mybir.SyncInfo(on_wait=[w], on_update=[]),
                        )
                        nc.register_instruction(nop, overwrite=True)
                        il.insert(i + j, nop)
                    inst.sync_info = mybir.SyncInfo(
                        on_wait=[waits[-1]], on_update=list(si.on_update)
                    )
                    i += len(waits) - 1
                i += 1


def build_wmat() -> np.ndarray:
    """Stationary matrices for the PE, [128, 512] f32, four [128,128] blocks.

    Input tiles are halo-aligned (partition k = output row r0+k-1); PSUM
    outputs are output-aligned (partition m = output row r0+m), so every
    block maps input partition m+1 -> output partition m.

    B0: vertical+center stencil, col m: 1 at m, -4 at m+1, 1 at m+2
    B1: row-shift selector, col m: 1 at m+1
    B2: 2 * B1
    B3: -B1
    """
    tv = np.zeros((128, 128), np.float32)
    sh = np.zeros((128, 128), np.float32)
    for m in range(126):
        tv[m, m] = 1.0
        tv[m + 1, m] = -4.0
        tv[m + 2, m] = 1.0
        sh[m + 1, m] = 1.0
    return np.concatenate([tv, sh, 2.0 * sh, -sh], axis=1)


def build_nc(repeats: int = 1) -> bass.Bass:
    nc = bass.Bass()
    p1s = nc.dram_tensor("p1s", [RPC + 2, EXT], F32, kind="ExternalInput")
    p2s = nc.dram_tensor("p2s", [RPC + 2, EXT], F32, kind="ExternalInput")
    pmls = nc.dram_tensor("pmls", [RPC + 2, EXT], F32, kind="ExternalInput")
    vs = nc.dram_tensor("vs", [RPC, NX], F32, kind="ExternalInput")
    wmat = nc.dram_tensor("wmat", [128, 512], F32, kind="ExternalInput")
    outd = nc.dram_tensor("outd", [RPC, 2 * SBW], F32, kind="ExternalOutput")

    with TileContext(nc) as tc:
        with (
            tc.tile_pool(name="const", bufs=1) as cpool,
            tc.tile_pool(name="io", bufs=2) as iopool,
            tc.tile_pool(name="work", bufs=3) as wpool,
            tc.tile_pool(name="psum", bufs=3, space=bass.MemorySpace.PSUM) as ppool,
        ):
            wt = cpool.tile([128, 512], F32)
            nc.sync.dma_start(out=wt[:], in_=wmat[:])

            for _rep in range(repeats):
                for band in range(NBANDS):
                    nr = BAND if band < NBANDS - 1 else RPC - BAND * (NBANDS - 1)
                    pr = nr + 2
                    r0 = band * BAND
                    for sb in range(2):
                        cg = SBW * sb
                        p2t = iopool.tile([128, SBW + 2], F32, tag="p2t")
                        pmlt = iopool.tile([128, SBW + 2], F32, tag="pmlt")
                        p1t = iopool.tile([128, SBW], F32, tag="p1t")
                        vt = iopool.tile([128, SBW], F32, tag="vt")
                        outt = iopool.tile([128, SBW], F32, tag="outt")
                        nc.sync.dma_start(
                            out=p2t[:pr], in_=p2s[r0:r0 + pr, 7 + cg:9 + cg + SBW]
                        )
                        nc.sync.dma_start(
                            out=pmlt[:pr], in_=pmls[r0:r0 + pr, 7 + cg:9 + cg + SBW]
                        )
                        nc.sync.dma_start(
                            out=p1t[:pr], in_=p1s[r0:r0 + pr, 8 + cg:8 + cg + SBW]
                        )
                        # varray slab with 32 edge-replicated columns at the
                        # outer end (global cols clip to [0, NX) after -40).
                        if sb == 0:
                            nc.sync.dma_start(
                                out=vt[:nr, 32:SBW],
                                in_=vs[r0:r0 + nr, 0:SBW - 32],
                            )
                            nc.scalar.activation(
                                out=vt[:nr, 0:32],
                                in_=vt[:nr, 32:64],
                                func=mybir.ActivationFunctionType.Identity,
                                bias=vt[:nr, 32:33],
                                scale=0.0,
                            )
                        else:
                            nc.sync.dma_start(
                                out=vt[:nr, 0:SBW - 32],
                                in_=vs[r0:r0 + nr, NX - (SBW - 32):NX],
                            )
                            nc.scalar.activation(
                                out=vt[:nr, SBW - 32:SBW],
                                in_=vt[:nr, 0:32],
                                func=mybir.ActivationFunctionType.Identity,
                                bias=vt[:nr, SBW - 33:SBW - 32],
                                scale=0.0,
                            )
                        for ch in range(4):
                            cc = CH * ch
                            p2c = wpool.tile([128, CH + 2], F32, tag="p2c")
                            nc.vector.tensor_mul(
                                out=p2c[:pr],
                                in0=p2t[:pr, cc:cc + CH + 2],
                                in1=pmlt[:pr, cc:cc + CH + 2],
                            )
                            # p1c on the same halo alignment (GpSimd, off DVE)
                            m1 = wpool.tile([128, CH], F32, tag="m1")
                            nc.gpsimd.tensor_mul(
                                out=m1[:pr],
                                in0=p1t[:pr, cc:cc + CH],
                                in1=pmlt[:pr, cc + 1:cc + 1 + CH],
                            )
                            # ps1 = 5-point Laplacian of p2c (center coeff -4)
                            ps1 = ppool.tile([128, CH], F32, tag="ps1")
                            nc.tensor.matmul(
                                ps1[:], wt[0:pr, 0:128], p2c[:pr, 1:1 + CH],
                                start=True, stop=False,
                            )
                            nc.tensor.matmul(
                                ps1[:], wt[0:pr, 128:256], p2c[:pr, 0:CH],
                                start=False, stop=False,
                            )
                            nc.tensor.matmul(
                                ps1[:], wt[0:pr, 128:256], p2c[:pr, 2:2 + CH],
                                start=False, stop=True,
                            )
                            # ps2 = 2*p2c - p1c   (both at output alignment)
                            ps2 = ppool.tile([128, CH], F32, tag="ps2")
                            nc.tensor.matmul(
                                ps2[:], wt[0:pr, 256:384], p2c[:pr, 1:1 + CH],
                                start=True, stop=False,
                            )
                            nc.tensor.matmul(
                                ps2[:], wt[0:pr, 384:512], m1[:pr],
                                start=False, stop=True,
                            )
                            # alpha = (v * DT/DX)^2
                            at = wpool.tile([128, CH], F32, tag="at")
                            nc.scalar.activation(
                                out=at[:nr],
                                in_=vt[:nr, cc:cc + CH],
                                func=mybir.ActivationFunctionType.Square,
                                scale=DT / DX,
                            )
                            s = wpool.tile([128, CH], F32, tag="s")
                            nc.vector.tensor_mul(
                                out=s[:nr], in0=ps1[:nr], in1=at[:nr]
                            )
                            nc.vector.tensor_add(
                                out=outt[:nr, cc:cc + CH],
                                in0=s[:nr],
                                in1=ps2[:nr],
                            )
                        nc.sync.dma_start(
                            out=outd[r0:r0 + nr, cg:cg + SBW], in_=outt[:nr, :]
                        )
    split_sync_waits(nc)
    return nc


def make_in_maps(p1, p2, pml_coeff, varray):
    """Split full inputs into per-core SPMD input maps (row sharding)."""
    p1 = np.ascontiguousarray(np.asarray(p1, dtype=np.float32))
    p2 = np.ascontiguousarray(np.asarray(p2, dtype=np.float32))
    pml = np.ascontiguousarray(np.asarray(pml_coeff, dtype=np.float32))
    v = np.ascontiguousarray(np.asarray(varray, dtype=np.float32))
    wmat = build_wmat()

    in_maps = []
    for k in range(NCORES):
        r0 = 8 + RPC * k          # first interior output row (global index)
        if k == 0:
            vs_k = np.concatenate(
                [np.broadcast_to(v[0:1], (32, NX)), v[0:RPC - 32]], axis=0
            )
        elif k == NCORES - 1:
            vs_k = np.concatenate(
                [v[r0 - 40:NX], np.broadcast_to(v[NX - 1:NX], (32, NX))], axis=0
            )
        else:
            vs_k = v[r0 - 40:r0 - 40 + RPC]
        in_maps.append({
            "p1s": p1[r0 - 1:r0 + RPC + 1],
            "p2s": p2[r0 - 1:r0 + RPC + 1],
            "pmls": pml[r0 - 1:r0 + RPC + 1],
            "vs": np.ascontiguousarray(vs_k),
            "wmat": wmat,
        })
    return in_maps


_NC_CACHE = {}


def _get_nc(repeats: int = 1) -> bass.Bass:
    if repeats not in _NC_CACHE:
        _NC_CACHE[repeats] = build_nc(repeats)
    return _NC_CACHE[repeats]


_last_results = None


def kernel(p1, p2, pml_coeff, varray, source_function, t):
    """Full-input entry point: shard across 8 cores, run, reassemble."""
    global _last_results
    in_maps = make_in_maps(p1, p2, pml_coeff, varray)
    nc = _get_nc(1)
    res = run_bass_kernel_spmd(nc, in_maps, core_ids=list(range(NCORES)))
    _last_results = res

    sf = np.asarray(source_function, dtype=np.float32)
    t = int(np.asarray(t))
    p_full = np.zeros((EXT, EXT), np.float32)
    interior = np.concatenate([res.results[k]["outd"] for k in range(NCORES)], axis=0)
    p_full[8:EXT - 8, 8:EXT - 8] = interior
    # scalar source injection at (SRC_X+PML, SRC_Y+PML) = (40, 60)
    p_full[40, 60] += np.float32(sf[t] * (DT * DT))
    return p_full, p_full[PML:EXT - PML, PML + 50].copy()
